# revision 5
# baseline (speedup 1.0000x reference)
"""Char-LSTM kernel for Trainium2 (8 NeuronCores, data parallel).

Strategy (v2)
-------------
Host side:
  * Sort words by length into per-core blocks of 512 words of a single
    length (per core there is exactly one block per length 1..16); leftovers
    fold into the length-16 block which then captures h every step.
  * Gather x_t = emb[chars[:, t]] on the host into per-step slabs
    [E=32 rows + bias-row 1 + zero pad], bf16.
  * All gate math is expressed through tanh only: sigmoid(x) = (1+tanh(x/2))/2.
    The 0.5 pre-scale for the i,f,o banks is folded into the weights, as is
    the 0.5 that converts the doubled state h2 = 2h back to h.  The device
    therefore computes, per step and bank column q:
        raw_q = Wq.T @ [h2 ; x ; 1]        (one K<=128 matmul, M=64)
        T = tanh(raw)                       (ONE activation over all 4 banks)
    and the cell update with state m = 2c:
        v  = (T_i + 1) * g~                 (= 2 sigmoid(i) tanh(g))
        u  = (T_f + 1) * m                  (= 2 sigmoid(f) m)
        m' = 0.5*u + v                      (= 2 c')
        tc = tanh(0.5 * m')
        h2 = (T_o + 1) * tc                 (= 2 h')
    each a single fused scalar_tensor_tensor on the Vector engine.
    The final output is h = h2/2, applied on the host after gather.
  * Block A lives on partitions 0:64, block B on 64:128 (mirrored slab
    layout so every elementwise op is partition-aligned).

Engine budget per group-step (both halves): 8 matmuls (N=512),
1 activation [128,2048] + 1 [128,512], 5 fused vector ops.
"""

import os
import sys

for _p in ("/opt/trn_rl_repo", "/root/.axon_site/_ro/trn_rl_repo"):
    if os.path.isdir(_p) and _p not in sys.path:
        sys.path.insert(0, _p)

import numpy as np
import ml_dtypes

BF16 = ml_dtypes.bfloat16

H = 64          # hidden size
E = 32          # char embedding size
V = 100         # vocab
MAXL = 16       # max word length
BLK = 512       # words per block (one half of a group)
NCORES = 8
GATE4 = 4 * H   # 256
XROWS = E + 1   # x slab rows: 32 emb dims + bias row

# torch gate order in the weights is [i, f, g, o]; we stage banks as
# [i, f, o, g] so the vector-op slices are contiguous per gate.
_GATE_PERM = np.concatenate([
    np.arange(0, 64),        # i
    np.arange(64, 128),      # f
    np.arange(192, 256),     # o
    np.arange(128, 192),     # g
])
# tanh pre-scale per staged bank: sigmoid banks (i,f,o) get 0.5, g gets 1.
_BANK_SCALE = np.repeat([0.5, 0.5, 0.5, 1.0], 64)

INTERLEAVE = int(os.environ.get("LSTM_INTERLEAVE", "3"))
M_F32 = os.environ.get("LSTM_M_F32", "1") == "1"
_PROGRAM_CACHE = {}


# --------------------------------------------------------------------------
# Host-side planning (unchanged from v1)
# --------------------------------------------------------------------------

def _plan(lengths):
    """Assign words to (core, block, column) slots."""
    n = lengths.shape[0]
    lengths = lengths.astype(np.int64)
    order = np.argsort(lengths, kind="stable")

    per_core_words = [[] for _ in range(NCORES)]
    block_meta = []

    leftovers = []
    for L in range(1, MAXL + 1):
        idx = order[np.searchsorted(lengths, L, side="left", sorter=order):
                    np.searchsorted(lengths, L, side="right", sorter=order)]
        take = idx[: NCORES * BLK]
        leftovers.append(idx[NCORES * BLK:])
        arr = np.full(NCORES * BLK, -1, dtype=np.int64)
        arr[: take.shape[0]] = take
        arr = arr.reshape(NCORES, BLK)
        for c in range(NCORES):
            per_core_words[c].append(arr[c])
        block_meta.append((L, False))

    leftovers = np.concatenate(leftovers) if leftovers else np.empty(0, np.int64)

    l16 = MAXL - 1
    free16 = [int((per_core_words[c][l16] < 0).sum()) for c in range(NCORES)]
    if leftovers.shape[0] <= sum(free16):
        block_meta[l16] = (MAXL, True)
        pos = 0
        for c in range(NCORES):
            k = min(free16[c], leftovers.shape[0] - pos)
            if k > 0:
                arr = per_core_words[c][l16]
                slots = np.nonzero(arr < 0)[0][:k]
                arr[slots] = leftovers[pos:pos + k]
                pos += k
        leftovers = leftovers[:0]

    if leftovers.shape[0]:
        n_ov = -(-leftovers.shape[0] // (NCORES * BLK))
        ov = np.full(n_ov * NCORES * BLK, -1, dtype=np.int64)
        ov[: leftovers.shape[0]] = leftovers
        ov = ov.reshape(n_ov, NCORES, BLK)
        for i in range(n_ov):
            for c in range(NCORES):
                per_core_words[c].append(ov[i, c])
            block_meta.append((MAXL, True))

    if len(block_meta) % 2 == 1:
        for c in range(NCORES):
            per_core_words[c].append(np.full(BLK, -1, dtype=np.int64))
        block_meta.append((1, False))

    nb = len(block_meta)
    key = sorted(range(nb), key=lambda b: (-block_meta[b][0], not block_meta[b][1]))
    blocks = []
    ov_count = 0
    for b in key:
        L, is_ov = block_meta[b]
        blocks.append({"L": L, "is_ov": is_ov,
                       "ov_idx": (ov_count if is_ov else -1), "orig": b})
        if is_ov:
            ov_count += 1

    assign = [[per_core_words[c][blocks[i]["orig"]] for i in range(nb)]
              for c in range(NCORES)]

    groups = []
    for i in range(0, nb, 2):
        groups.append({"a": i, "b": i + 1,
                       "steps": max(blocks[i]["L"], blocks[i + 1]["L"])})

    remaining = [g["steps"] for g in groups]
    next_t = [0] * len(groups)
    sched = []
    while any(r > 0 for r in remaining):
        act = sorted(range(len(groups)), key=lambda g: -remaining[g])[:INTERLEAVE]
        act = [g for g in act if remaining[g] > 0]
        for g in act:
            sched.append((g, next_t[g]))
            next_t[g] += 1
            remaining[g] -= 1

    for bi, blk in enumerate(blocks):
        if not blk["is_ov"]:
            continue
        steps = set()
        for c in range(NCORES):
            w = assign[c][bi]
            w = w[w >= 0]
            steps.update((lengths[w] - 1).tolist())
        blk["cap_steps"] = tuple(sorted(steps))

    return {"blocks": blocks, "groups": groups, "sched": sched,
            "assign": assign, "n_ov": ov_count}


def _build_xg(plan, chars, lengths, emb_s):
    """Per-core x-gather slabs [n_slabs, 64, BLK] bf16.

    Rows 0:32 = emb[ch].T for alive words (0 otherwise), row 32 = 1.0
    (bias row), rows 33:64 = 0.  Slab order: for each scheduled (group, t):
    A half then B half.
    """
    blocks, groups, sched = plan["blocks"], plan["groups"], plan["sched"]
    n_slabs = 2 * len(sched)
    out = []
    for c in range(NCORES):
        xg = np.zeros((n_slabs, 64, BLK), dtype=BF16)
        xg[:, E, :] = 1.0
        slab = 0
        for (g, t) in sched:
            for blk_idx in (groups[g]["a"], groups[g]["b"]):
                words = plan["assign"][c][blk_idx]
                valid = (words >= 0)
                w = words[valid]
                if w.shape[0]:
                    alive = t < lengths[w]
                    cols = np.nonzero(valid)[0][alive]
                    ch = chars[w[alive], t]
                    xg[slab, 0:E, cols] = emb_s[ch]
                slab += 1
        out.append(xg)
    return out


# --------------------------------------------------------------------------
# Device program
# --------------------------------------------------------------------------

def _build_program(plan_sig, blocks, groups, sched, n_ov, variant="full",
                   reps=1):
    import concourse.bass as bass
    import concourse.tile as tile
    from concourse import bacc, mybir
    from contextlib import nullcontext

    do_mm = variant not in ("nomm",)
    do_act = variant not in ("noact",)
    do_dma = variant not in ("nodma",)

    f32 = mybir.dt.float32
    bf16 = mybir.dt.bfloat16
    mdt = f32 if M_F32 else bf16
    ADD = mybir.AluOpType.add
    MUL = mybir.AluOpType.mult
    TANH = mybir.ActivationFunctionType.Tanh
    n_blocks = len(blocks)
    n_slabs = 2 * len(sched)

    nc = bacc.Bacc("TRN2", target_bir_lowering=False, debug=False,
                   num_devices=NCORES)
    xg_d = nc.dram_tensor("xg", [n_slabs, 64, BLK], bf16, kind="ExternalInput")
    wa_d = nc.dram_tensor("wa", [128, GATE4], bf16, kind="ExternalInput")
    wb_d = nc.dram_tensor("wb", [128, GATE4], bf16, kind="ExternalInput")
    out_d = nc.dram_tensor("out", [n_blocks, H, BLK], f32, kind="ExternalOutput")
    ov_d = nc.dram_tensor("ov", [max(1, n_ov) * MAXL, H, BLK], f32,
                          kind="ExternalOutput")

    with tile.TileContext(nc) as tc:
        with (
            tc.tile_pool(name="consts", bufs=1) as consts,
            tc.tile_pool(name="slabs", bufs=18) as slabs,
            tc.tile_pool(name="psum", bufs=2, space="PSUM") as psump,
            tc.tile_pool(name="tpool", bufs=3) as tpool,
            tc.tile_pool(name="tcp", bufs=3) as tcp,
            tc.tile_pool(name="vp", bufs=3) as vp,
            tc.tile_pool(name="up", bufs=3) as up,
            tc.tile_pool(name="state", bufs=8) as statep,
            tc.tile_pool(name="hfp", bufs=3) as hfp,
        ):
            wa = consts.tile([128, GATE4], bf16, tag="wa")
            wb = consts.tile([128, GATE4], bf16, tag="wb")
            nc.sync.dma_start(out=wa[:], in_=wa_d[:])
            nc.sync.dma_start(out=wb[:], in_=wb_d[:])

            sched_pos = {gt: j for j, gt in enumerate(sched)}

            loop_cm = tc.For_i(0, reps, 1) if reps > 1 else nullcontext()
            with loop_cm:
                gstate = {}
                for (g, t) in sched:
                    grp = groups[g]
                    a, b = blocks[grp["a"]], blocks[grp["b"]]
                    La, Lb = a["L"], b["L"]
                    b_alive = t < Lb
                    sl = slice(0, 128 if b_alive else 64)
                    cur = 2 * sched_pos[(g, t)]

                    st = gstate.get(g)
                    if t == 0:
                        # step-0 slabs (h2 part zeroed); later slabs are
                        # allocated one step ahead for the h2 write.
                        sA = slabs.tile([128, BLK], bf16, tag="slab", name="sA0")
                        nc.gpsimd.memset(sA[0:64, :], 0.0)
                        if do_dma:
                            nc.sync.dma_start(out=sA[64:64 + XROWS, :],
                                              in_=xg_d[cur, 0:XROWS])
                        sB = slabs.tile([128, BLK], bf16, tag="slab", name="sB0")
                        nc.gpsimd.memset(sB[64:128, :], 0.0)
                        if do_dma:
                            nc.sync.dma_start(out=sB[0:64, :],
                                              in_=xg_d[cur + 1, 0:64])
                        st = gstate[g] = {
                            "sA": sA, "sB": sB,
                            "m": statep.tile([128, BLK], mdt, tag="m", name="m"),
                        }

                    sA, sB = st["sA"], st["sB"]

                    # --- matmuls: raw gates into one [128, 2048] PSUM tile
                    ps = psump.tile([128, 4 * BLK], f32, tag="ps")
                    if do_mm:
                        for q in range(4):
                            qs = slice(64 * q, 64 * q + 64)
                            cs = slice(BLK * q, BLK * q + BLK)
                            nc.tensor.matmul(ps[0:64, cs], wa[0:64 + XROWS, qs],
                                             sA[0:64 + XROWS, :],
                                             start=True, stop=True,
                                             tile_position=(0, 0))
                            if b_alive:
                                nc.tensor.matmul(ps[64:128, cs], wb[:, qs],
                                                 sB[:, :],
                                                 start=True, stop=True,
                                                 tile_position=(0, 64))

                    # --- next-step slabs (allocated now so h2 can be written)
                    a_next = t + 1 < La
                    b_next = t + 1 < Lb
                    if a_next or b_next:
                        nxt = 2 * sched_pos[(g, t + 1)]
                    if a_next:
                        sA2 = slabs.tile([128, BLK], bf16, tag="slab", name="sA")
                        if do_dma:
                            nc.sync.dma_start(out=sA2[64:64 + XROWS, :],
                                              in_=xg_d[nxt, 0:XROWS])
                        st["sA"] = sA2
                    if b_next:
                        sB2 = slabs.tile([128, BLK], bf16, tag="slab", name="sB")
                        if do_dma:
                            nc.sync.dma_start(out=sB2[0:64, :],
                                              in_=xg_d[nxt + 1, 0:64])
                        st["sB"] = sB2

                    if not do_act:
                        continue

                    # --- activations + cell update
                    T = tpool.tile([128, 4 * BLK], bf16, tag="T")
                    nc.scalar.activation(out=T[sl, :], in_=ps[sl, :], func=TANH)

                    Ti = T[sl, 0:BLK]
                    Tf = T[sl, BLK:2 * BLK]
                    To = T[sl, 2 * BLK:3 * BLK]
                    Tg = T[sl, 3 * BLK:4 * BLK]
                    m = st["m"]

                    if t == 0:
                        nc.vector.scalar_tensor_tensor(
                            out=m[sl, :], in0=Ti, scalar=1.0, in1=Tg,
                            op0=ADD, op1=MUL)
                    else:
                        v = vp.tile([128, BLK], bf16, tag="v")
                        u = up.tile([128, BLK], mdt, tag="u")
                        nc.vector.scalar_tensor_tensor(
                            out=v[sl, :], in0=Ti, scalar=1.0, in1=Tg,
                            op0=ADD, op1=MUL)
                        nc.vector.scalar_tensor_tensor(
                            out=u[sl, :], in0=Tf, scalar=1.0, in1=m[sl, :],
                            op0=ADD, op1=MUL)
                        nc.vector.scalar_tensor_tensor(
                            out=m[sl, :], in0=u[sl, :], scalar=0.5, in1=v[sl, :],
                            op0=MUL, op1=ADD)

                    tc_ = tcp.tile([128, BLK], bf16, tag="tc")
                    nc.scalar.activation(out=tc_[sl, :], in_=m[sl, :],
                                         func=TANH, scale=0.5)

                    # h2 = (T_o + 1) * tc -> next step's slab (bf16)
                    if a_next:
                        nc.vector.scalar_tensor_tensor(
                            out=st["sA"][0:64, :], in0=T[0:64, 2 * BLK:3 * BLK],
                            scalar=1.0, in1=tc_[0:64, :], op0=ADD, op1=MUL)
                    if b_next:
                        nc.vector.scalar_tensor_tensor(
                            out=st["sB"][64:128, :], in0=T[64:128, 2 * BLK:3 * BLK],
                            scalar=1.0, in1=tc_[64:128, :], op0=ADD, op1=MUL)

                    # f32 h2 output (host halves it)
                    cap_a = a["is_ov"] and t in a.get("cap_steps", ())
                    cap_b = b["is_ov"] and t in b.get("cap_steps", ())
                    need_a = (t == La - 1) or cap_a
                    need_b = b_alive and ((t == Lb - 1) or cap_b)
                    if need_a or need_b:
                        hf = hfp.tile([128, BLK], f32, tag="hf", name="hf")
                        if need_a:
                            nc.vector.scalar_tensor_tensor(
                                out=hf[0:64, :], in0=T[0:64, 2 * BLK:3 * BLK],
                                scalar=1.0, in1=tc_[0:64, :], op0=ADD, op1=MUL)
                            if t == La - 1:
                                nc.sync.dma_start(out=out_d[grp["a"]],
                                                  in_=hf[0:64, :])
                            if cap_a:
                                nc.sync.dma_start(
                                    out=ov_d[a["ov_idx"] * MAXL + t],
                                    in_=hf[0:64, :])
                        if need_b:
                            nc.vector.scalar_tensor_tensor(
                                out=hf[64:128, :], in0=T[64:128, 2 * BLK:3 * BLK],
                                scalar=1.0, in1=tc_[64:128, :], op0=ADD, op1=MUL)
                            if t == Lb - 1:
                                nc.sync.dma_start(out=out_d[grp["b"]],
                                                  in_=hf[64:128, :])
                            if cap_b:
                                nc.sync.dma_start(
                                    out=ov_d[b["ov_idx"] * MAXL + t],
                                    in_=hf[64:128, :])

    nc.compile()
    return nc


# --------------------------------------------------------------------------
# Entry point
# --------------------------------------------------------------------------

def kernel(emb, W_ih, W_hh, b_ih, b_hh, chars, lengths):
    from concourse.bass_utils import run_bass_kernel_spmd

    emb = np.asarray(emb, dtype=np.float32)
    W_ih = np.asarray(W_ih, dtype=np.float32)
    W_hh = np.asarray(W_hh, dtype=np.float32)
    b_ih = np.asarray(b_ih, dtype=np.float32)
    b_hh = np.asarray(b_hh, dtype=np.float32)
    chars = np.asarray(chars)
    lengths_np = np.asarray(lengths)

    n = chars.shape[0]

    # --- weight prep -------------------------------------------------------
    # Staged bank layout [i, f, o, g]; tanh pre-scale on sigmoid banks and
    # the h2 = 2h halving are folded in here.
    s = _BANK_SCALE                                     # [256]
    Wh = (0.5 * W_hh.T[:, _GATE_PERM]) * s              # [64, 256]
    Wx = W_ih.T[:, _GATE_PERM] * s                      # [32, 256]
    bias = ((b_ih + b_hh)[_GATE_PERM] * s)[None, :]     # [1, 256]

    wA = np.zeros((128, GATE4), dtype=BF16)
    wA[0:64] = Wh.astype(BF16)
    wA[64:96] = Wx.astype(BF16)
    wA[96:97] = bias.astype(BF16)

    wB = np.zeros((128, GATE4), dtype=BF16)
    wB[0:32] = Wx.astype(BF16)
    wB[32:33] = bias.astype(BF16)
    wB[64:128] = Wh.astype(BF16)

    # --- word assignment ---------------------------------------------------
    plan = _plan(lengths_np)
    blocks, groups, sched = plan["blocks"], plan["groups"], plan["sched"]

    sig = (tuple((b["L"], b["is_ov"], b.get("cap_steps", ())) for b in blocks),
           tuple(sched))
    key = hash(sig)
    if key not in _PROGRAM_CACHE:
        _PROGRAM_CACHE[key] = _build_program(sig, blocks, groups, sched,
                                             plan["n_ov"])
    nc = _PROGRAM_CACHE[key]

    xgs = _build_xg(plan, chars, lengths_np, emb.astype(BF16))
    in_maps = [{"xg": xgs[c], "wa": wA, "wb": wB} for c in range(NCORES)]

    res = run_bass_kernel_spmd(nc, in_maps, core_ids=list(range(NCORES)))
    kernel._last_nc = nc
    kernel._last_in_maps = in_maps

    # --- gather results ----------------------------------------------------
    outs = np.stack([r["out"] for r in res.results])    # [8, nb, H, BLK] = 2h
    ovs = np.stack([r["ov"] for r in res.results])      # [8, n_ov*16, H, BLK]

    result = np.empty((n, H), dtype=np.float32)
    for c in range(NCORES):
        for bi, blk in enumerate(blocks):
            words = plan["assign"][c][bi]
            valid = words >= 0
            if not valid.any():
                continue
            w = words[valid]
            cols = np.nonzero(valid)[0]
            if blk["is_ov"]:
                steps = lengths_np[w].astype(np.int64) - 1
                result[w] = ovs[c, blk["ov_idx"] * MAXL + steps, :, cols]
            else:
                result[w] = outs[c, bi, :, cols]
    result *= 0.5
    return result


# revision 10
# speedup vs baseline: 1.9433x; 1.9433x over previous
"""Char-LSTM kernel for Trainium2 (8 NeuronCores, data parallel).

Strategy (v3)
-------------
Vocab is only 100, so the LSTM state after 1 char has 100 distinct values and
after 2 chars 10^4 — both computed exactly on the host in fp32:
  * words of length <= 2 never touch the device (table lookup), and
  * device blocks start at absolute step 2 with DMA'd initial (c, h),
cutting device steps from sum(L) to sum(L-2).

Device algorithm per step (block A on partitions 0:64, B on 64:128):
  * raw gates via 8 matmuls (one per bank and half) on concat slabs
    [h ; x=emb[ch] ; 1] with per-bank tanh pre-scaling folded into weights
    (sigmoid(x) = (1+tanh(x/2))/2, so the i,f,o weight columns carry 0.5).
  * ONE activation: T = tanh(raw) over all four banks [128, 2048].
  * P = (T_ifo + 1) * 0.5  — one tensor_scalar op (4x bf16 mode) giving the
    three sigmoids; then plain tensor_tensor bf16 ops (2x mode):
      v = P_i * g~ ; u = P_f * c ; c' = u + v ; tc = tanh(c') [ACT] ;
      h = P_o * tc  (written straight into the next step's slab).
Words are sorted by length into single-length blocks of 512, paired into
groups; leftovers fold into the longest block which captures h every step.
"""

import os
import sys

for _p in ("/opt/trn_rl_repo", "/root/.axon_site/_ro/trn_rl_repo"):
    if os.path.isdir(_p) and _p not in sys.path:
        sys.path.insert(0, _p)

import numpy as np
import ml_dtypes

BF16 = ml_dtypes.bfloat16

H = 64          # hidden size
E = 32          # char embedding size
V = 100         # vocab
MAXL = 16       # max word length
SKIP = 2        # steps resolved by host tables
DEVL = MAXL - SKIP
BLK = 512       # words per block (one half of a group)
NCORES = 8
GATE4 = 4 * H   # 256
XROWS = E + 1   # x slab rows: 32 emb dims + bias row

# torch gate order in the weights is [i, f, g, o]; staged as [i, f, o, g].
_GATE_PERM = np.concatenate([
    np.arange(0, 64),        # i
    np.arange(64, 128),      # f
    np.arange(192, 256),     # o
    np.arange(128, 192),     # g
])
_BANK_SCALE = np.repeat([0.5, 0.5, 0.5, 1.0], 64)   # tanh pre-scale

INTERLEAVE = int(os.environ.get("LSTM_INTERLEAVE", "3"))
C_F32 = os.environ.get("LSTM_C_F32", "0") == "1"
_PROGRAM_CACHE = {}


# --------------------------------------------------------------------------
# Host-side planning
# --------------------------------------------------------------------------

def _plan(lengths):
    """Assign device words (len > SKIP) to (core, block, column) slots.

    All device words are sorted by dev length (lengths-SKIP) descending and
    dealt round-robin across cores, then chopped into 512-word blocks, so
    block k holds the globally k-th longest span of words.  Every block
    captures h at each of its words' final steps (cap_steps), the result is
    read from the ov buffer at step dev_len-1.
    """
    lengths = np.asarray(lengths).astype(np.int64)
    dev_len = lengths - SKIP

    ids = np.nonzero(dev_len >= 1)[0]
    ids = ids[np.argsort(-dev_len[ids], kind="stable")]
    n_dev = ids.shape[0]

    nb = -(-n_dev // (NCORES * BLK))
    if nb % 2:
        nb += 1
    dealt = np.full(nb * NCORES * BLK, -1, dtype=np.int64)
    dealt[:n_dev] = ids
    # rank r -> core r % NCORES, per-core slot r // NCORES
    percore = dealt.reshape(nb * BLK, NCORES).T        # [NCORES, nb*BLK]

    blocks = []
    assign = [[] for _ in range(NCORES)]
    for k in range(nb):
        caps = set()
        Lk = 1
        for c in range(NCORES):
            w = percore[c, k * BLK:(k + 1) * BLK]
            assign[c].append(w)
            wv = w[w >= 0]
            if wv.shape[0]:
                dl = dev_len[wv]
                Lk = max(Lk, int(dl.max()))
                caps.update((dl - 1).tolist())
        blocks.append({"L": Lk, "is_ov": True, "ov_idx": k,
                       "cap_steps": tuple(sorted(caps))})

    groups = []
    for i in range(0, nb, 2):
        groups.append({"a": i, "b": i + 1,
                       "steps": max(blocks[i]["L"], blocks[i + 1]["L"])})

    remaining = [g["steps"] for g in groups]
    next_t = [0] * len(groups)
    sched = []
    while any(r > 0 for r in remaining):
        act = sorted(range(len(groups)), key=lambda g: -remaining[g])[:INTERLEAVE]
        act = [g for g in act if remaining[g] > 0]
        for g in act:
            sched.append((g, next_t[g]))
            next_t[g] += 1
            remaining[g] -= 1

    return {"blocks": blocks, "groups": groups, "sched": sched,
            "assign": assign, "n_ov": nb}


def _host_tables(emb, W_ih, W_hh, b_ih, b_hh):
    """Exact fp32 LSTM states after 1 and 2 chars for all prefixes."""
    def sig(x):
        return 1.0 / (1.0 + np.exp(-x))

    G1 = emb @ W_ih.T + b_ih + b_hh                     # [V, 4H] i,f,g,o
    i1, f1, g1, o1 = np.split(G1, 4, axis=1)
    c1 = sig(i1) * np.tanh(g1)                          # [V, H]
    h1 = sig(o1) * np.tanh(c1)

    HW2 = h1 @ W_hh.T                                   # [V, 4H]
    G2 = G1[None, :, :] + HW2[:, None, :]               # [V(c0), V(c1), 4H]
    i2, f2, g2, o2 = np.split(G2, 4, axis=2)
    c2 = sig(f2) * c1[:, None, :] + sig(i2) * np.tanh(g2)   # [V, V, H]
    h2 = sig(o2) * np.tanh(c2)
    return h1, c2.reshape(V * V, H), h2.reshape(V * V, H)


def _build_inputs(plan, chars, lengths, emb_bf, c2, h2):
    """Per-core device input tensors.

    xg    [n_slabs, 64, BLK] bf16: rows 0:32 emb[ch] at absolute step t+SKIP,
          row 32 = 1.0, rest 0.  Slab order: sched x (A, B).
    cinit [n_blocks, 64, BLK] bf16, hinit likewise: state after 2 chars.
    """
    blocks, groups, sched = plan["blocks"], plan["groups"], plan["sched"]
    n_slabs = 2 * len(sched)
    nb = len(blocks)
    out = []
    for c in range(NCORES):
        xg = np.zeros((n_slabs, 64, BLK), dtype=BF16)
        xg[:, E, :] = 1.0
        slab = 0
        for (g, t) in sched:
            for blk_idx in (groups[g]["a"], groups[g]["b"]):
                words = plan["assign"][c][blk_idx]
                valid = (words >= 0)
                w = words[valid]
                if w.shape[0]:
                    alive = t + SKIP < lengths[w]
                    cols = np.nonzero(valid)[0][alive]
                    ch = chars[w[alive], t + SKIP]
                    xg[slab, 0:E, cols] = emb_bf[ch]
                slab += 1
        cinit = np.zeros((nb, H, BLK), dtype=BF16)
        hinit = np.zeros((nb, H, BLK), dtype=BF16)
        for bi in range(nb):
            words = plan["assign"][c][bi]
            valid = words >= 0
            w = words[valid]
            if not w.shape[0]:
                continue
            cols = np.nonzero(valid)[0]
            pair = chars[w, 0] * V + chars[w, 1]
            cinit[bi, :, cols] = c2[pair].astype(BF16)
            hinit[bi, :, cols] = h2[pair].astype(BF16)
        out.append({"xg": xg, "cinit": cinit, "hinit": hinit})
    return out


# --------------------------------------------------------------------------
# Device program
# --------------------------------------------------------------------------

def _build_program(plan_sig, blocks, groups, sched, n_ov, variant="full",
                   reps=1):
    import concourse.bass as bass
    import concourse.tile as tile
    from concourse import bacc, mybir
    from contextlib import nullcontext

    do_mm = variant not in ("nomm",)
    do_act = variant not in ("noact",)
    do_dma = variant not in ("nodma",)

    f32 = mybir.dt.float32
    bf16 = mybir.dt.bfloat16
    cdt = f32 if C_F32 else bf16
    ADD = mybir.AluOpType.add
    MUL = mybir.AluOpType.mult
    TANH = mybir.ActivationFunctionType.Tanh
    n_blocks = len(blocks)
    n_slabs = 2 * len(sched)

    nc = bacc.Bacc("TRN2", target_bir_lowering=False, debug=False,
                   num_devices=NCORES)
    xg_d = nc.dram_tensor("xg", [n_slabs, 64, BLK], bf16, kind="ExternalInput")
    ci_d = nc.dram_tensor("cinit", [n_blocks, H, BLK], bf16,
                          kind="ExternalInput")
    hi_d = nc.dram_tensor("hinit", [n_blocks, H, BLK], bf16,
                          kind="ExternalInput")
    wa_d = nc.dram_tensor("wa", [128, GATE4], bf16, kind="ExternalInput")
    wb_d = nc.dram_tensor("wb", [128, GATE4], bf16, kind="ExternalInput")
    ov_d = nc.dram_tensor("ov", [max(1, n_ov) * DEVL, H, BLK], f32,
                          kind="ExternalOutput")

    with tile.TileContext(nc) as tc:
        with (
            tc.tile_pool(name="consts", bufs=1) as consts,
            tc.tile_pool(name="slabs", bufs=18) as slabs,
            tc.tile_pool(name="psum", bufs=2, space="PSUM") as psump,
            tc.tile_pool(name="tpool", bufs=3) as tpool,
            tc.tile_pool(name="ppool", bufs=3) as ppool,
            tc.tile_pool(name="tcp", bufs=3) as tcp,
            tc.tile_pool(name="vp", bufs=3) as vp,
            tc.tile_pool(name="up", bufs=3) as up,
            tc.tile_pool(name="state", bufs=8) as statep,
            tc.tile_pool(name="hfp", bufs=3) as hfp,
        ):
            wa = consts.tile([128, GATE4], bf16, tag="wa")
            wb = consts.tile([128, GATE4], bf16, tag="wb")
            nc.sync.dma_start(out=wa[:], in_=wa_d[:])
            nc.sync.dma_start(out=wb[:], in_=wb_d[:])

            sched_pos = {gt: j for j, gt in enumerate(sched)}

            loop_cm = tc.For_i(0, reps, 1) if reps > 1 else nullcontext()
            with loop_cm:
                gstate = {}
                for (g, t) in sched:
                    grp = groups[g]
                    a, b = blocks[grp["a"]], blocks[grp["b"]]
                    La, Lb = a["L"], b["L"]
                    b_alive = t < Lb
                    sl = slice(0, 128 if b_alive else 64)
                    cur = 2 * sched_pos[(g, t)]

                    st = gstate.get(g)
                    if t == 0:
                        sA = slabs.tile([128, BLK], bf16, tag="slab", name="sA0")
                        sB = slabs.tile([128, BLK], bf16, tag="slab", name="sB0")
                        cst = statep.tile([128, BLK], cdt, tag="c", name="c")
                        if do_dma:
                            nc.sync.dma_start(out=sA[0:64, :],
                                              in_=hi_d[grp["a"]])
                            nc.sync.dma_start(out=sA[64:64 + XROWS, :],
                                              in_=xg_d[cur, 0:XROWS])
                            nc.sync.dma_start(out=sB[64:128, :],
                                              in_=hi_d[grp["b"]])
                            nc.sync.dma_start(out=sB[0:64, :],
                                              in_=xg_d[cur + 1, 0:64])
                            nc.sync.dma_start(out=cst[0:64, :],
                                              in_=ci_d[grp["a"]])
                            nc.sync.dma_start(out=cst[64:128, :],
                                              in_=ci_d[grp["b"]])
                        st = gstate[g] = {"sA": sA, "sB": sB, "c": cst}

                    sA, sB = st["sA"], st["sB"]

                    # --- matmuls: raw gates into one [128, 2048] PSUM tile
                    ps = psump.tile([128, 4 * BLK], f32, tag="ps")
                    if do_mm:
                        for q in range(4):
                            qs = slice(64 * q, 64 * q + 64)
                            cs = slice(BLK * q, BLK * q + BLK)
                            nc.tensor.matmul(ps[0:64, cs], wa[0:64 + XROWS, qs],
                                             sA[0:64 + XROWS, :],
                                             start=True, stop=True,
                                             tile_position=(0, 0))
                            if b_alive:
                                nc.tensor.matmul(ps[64:128, cs], wb[:, qs],
                                                 sB[:, :],
                                                 start=True, stop=True,
                                                 tile_position=(0, 64))

                    # --- next-step slabs (allocated now so h can be written)
                    a_next = t + 1 < La
                    b_next = t + 1 < Lb
                    if a_next or b_next:
                        nxt = 2 * sched_pos[(g, t + 1)]
                    if a_next:
                        sA2 = slabs.tile([128, BLK], bf16, tag="slab", name="sA")
                        if do_dma:
                            nc.sync.dma_start(out=sA2[64:64 + XROWS, :],
                                              in_=xg_d[nxt, 0:XROWS])
                        st["sA"] = sA2
                    if b_next:
                        sB2 = slabs.tile([128, BLK], bf16, tag="slab", name="sB")
                        if do_dma:
                            nc.sync.dma_start(out=sB2[0:64, :],
                                              in_=xg_d[nxt + 1, 0:64])
                        st["sB"] = sB2

                    if not do_act:
                        continue

                    # --- activations + cell update
                    T = tpool.tile([128, 4 * BLK], bf16, tag="T")
                    nc.scalar.activation(out=T[sl, :], in_=ps[sl, :], func=TANH)

                    # P = (T_ifo + 1) * 0.5 -> sigmoids of i, f, o
                    P = ppool.tile([128, 3 * BLK], bf16, tag="P")
                    nc.vector.tensor_scalar(out=P[sl, :], in0=T[sl, 0:3 * BLK],
                                            scalar1=1.0, scalar2=0.5,
                                            op0=ADD, op1=MUL)

                    cst = st["c"]
                    v = vp.tile([128, BLK], bf16, tag="v")
                    u = up.tile([128, BLK], cdt, tag="u")
                    nc.vector.tensor_mul(v[sl, :], P[sl, 0:BLK],
                                         T[sl, 3 * BLK:4 * BLK])
                    nc.vector.tensor_mul(u[sl, :], P[sl, BLK:2 * BLK],
                                         cst[sl, :])
                    nc.vector.tensor_add(cst[sl, :], u[sl, :], v[sl, :])

                    tc_ = tcp.tile([128, BLK], bf16, tag="tc")
                    nc.scalar.activation(out=tc_[sl, :], in_=cst[sl, :],
                                         func=TANH)

                    # h = P_o * tc -> next step's slab (bf16)
                    if a_next:
                        nc.vector.tensor_mul(st["sA"][0:64, :],
                                             P[0:64, 2 * BLK:3 * BLK],
                                             tc_[0:64, :])
                    if b_next:
                        nc.vector.tensor_mul(st["sB"][64:128, :],
                                             P[64:128, 2 * BLK:3 * BLK],
                                             tc_[64:128, :])

                    # f32 h captures at word-final steps
                    need_a = t in a["cap_steps"]
                    need_b = b_alive and t in b["cap_steps"]
                    if need_a or need_b:
                        hf = hfp.tile([128, BLK], f32, tag="hf", name="hf")
                        if need_a:
                            nc.vector.tensor_mul(hf[0:64, :],
                                                 P[0:64, 2 * BLK:3 * BLK],
                                                 tc_[0:64, :])
                            nc.sync.dma_start(
                                out=ov_d[a["ov_idx"] * DEVL + t],
                                in_=hf[0:64, :])
                        if need_b:
                            nc.vector.tensor_mul(hf[64:128, :],
                                                 P[64:128, 2 * BLK:3 * BLK],
                                                 tc_[64:128, :])
                            nc.sync.dma_start(
                                out=ov_d[b["ov_idx"] * DEVL + t],
                                in_=hf[64:128, :])

    nc.compile()
    return nc


# --------------------------------------------------------------------------
# Entry point
# --------------------------------------------------------------------------

def kernel(emb, W_ih, W_hh, b_ih, b_hh, chars, lengths):
    from concourse.bass_utils import run_bass_kernel_spmd

    emb = np.asarray(emb, dtype=np.float32)
    W_ih = np.asarray(W_ih, dtype=np.float32)
    W_hh = np.asarray(W_hh, dtype=np.float32)
    b_ih = np.asarray(b_ih, dtype=np.float32)
    b_hh = np.asarray(b_hh, dtype=np.float32)
    chars = np.asarray(chars)
    lengths_np = np.asarray(lengths).astype(np.int64)

    n = chars.shape[0]

    # --- host prefix tables ------------------------------------------------
    h1, c2, h2 = _host_tables(emb, W_ih, W_hh, b_ih, b_hh)

    # --- weight prep -------------------------------------------------------
    s = _BANK_SCALE
    Wh = (W_hh.T[:, _GATE_PERM]) * s                    # [64, 256]
    Wx = W_ih.T[:, _GATE_PERM] * s                      # [32, 256]
    bias = ((b_ih + b_hh)[_GATE_PERM] * s)[None, :]     # [1, 256]

    wA = np.zeros((128, GATE4), dtype=BF16)
    wA[0:64] = Wh.astype(BF16)
    wA[64:96] = Wx.astype(BF16)
    wA[96:97] = bias.astype(BF16)

    wB = np.zeros((128, GATE4), dtype=BF16)
    wB[0:32] = Wx.astype(BF16)
    wB[32:33] = bias.astype(BF16)
    wB[64:128] = Wh.astype(BF16)

    # --- word assignment ---------------------------------------------------
    plan = _plan(lengths_np)
    blocks, groups, sched = plan["blocks"], plan["groups"], plan["sched"]

    sig = (tuple((b["L"], b["is_ov"], b.get("cap_steps", ())) for b in blocks),
           tuple(sched))
    key = hash(sig)
    if key not in _PROGRAM_CACHE:
        _PROGRAM_CACHE[key] = _build_program(sig, blocks, groups, sched,
                                             plan["n_ov"])
    nc = _PROGRAM_CACHE[key]

    percore = _build_inputs(plan, chars, lengths_np, emb.astype(BF16), c2, h2)
    in_maps = [{"xg": percore[c]["xg"], "cinit": percore[c]["cinit"],
                "hinit": percore[c]["hinit"], "wa": wA, "wb": wB}
               for c in range(NCORES)]

    res = run_bass_kernel_spmd(nc, in_maps, core_ids=list(range(NCORES)))
    kernel._last_nc = nc
    kernel._last_in_maps = in_maps

    # --- gather results ----------------------------------------------------
    ovs = np.stack([r["ov"] for r in res.results])      # [8, nb*DEVL, H, BLK]

    result = np.empty((n, H), dtype=np.float32)
    short1 = lengths_np == 1
    result[short1] = h1[chars[short1, 0]]
    short2 = lengths_np == 2
    result[short2] = h2[chars[short2, 0] * V + chars[short2, 1]]

    for c in range(NCORES):
        for bi, blk in enumerate(blocks):
            words = plan["assign"][c][bi]
            valid = words >= 0
            if not valid.any():
                continue
            w = words[valid]
            cols = np.nonzero(valid)[0]
            steps = lengths_np[w] - SKIP - 1
            result[w] = ovs[c, blk["ov_idx"] * DEVL + steps, :, cols]
    return result
